# revision 2
# baseline (speedup 1.0000x reference)
"""GCNII (64-layer) + MLP head on 8 Trainium2 NeuronCores.

Strategy (node-sharded graph parallel):
  - Nodes sharded contiguously across 8 cores (12500 each). Each core owns the
    segment-sum for its destination nodes.
  - The full node-feature table x (scaled by dinv[src]) is replicated in each
    core's DRAM ([100000, 64] f32, 256B rows) and refreshed per layer with an
    AllGather of the per-core shards.
  - Per layer, each core gathers its in-edges' source rows with dma_gather
    (int16 indices -> 4 source-range buckets of 32768 rows), aggregates them
    per 512-destination window via indicator matmuls into PSUM
    (psum[64, 512] += gathered[128e, 64f].T @ onehot[128e, 96]), applies the
    GCNII update h = 0.9*dinv[dst]*agg + 0.1*x0, x = relu(h @ W'l) with
    W'l = (1-b)I + b*Wl folded on the host, transposes back to node-major,
    rescales by dinv, and AllGathers the new shard.
  - PSUM column offsets are per-core data (loaded into PE registers from SBUF)
    so a single SPMD program serves all 8 cores; chunk-count templates are
    cross-core maxima.
"""
import os
import numpy as np

# problem dims
N, F, H, L, R, C, E = 100000, 500, 64, 64, 512, 40, 1000000
ALPHA, THETA = 0.1, 0.5
M1, M2 = (R - H) // 3 + H, 2 * ((R - H) // 3) + H  # 213, 362
NCORES = 8
NP = N // NCORES          # 12500
WIN = 512                 # dst window width (psum bank free dim)
NWIN = (NP + WIN - 1) // WIN   # 25 (last window 212 wide)
WBLOCK = 3                # windows per gather-call block
NBLK = (NWIN + WBLOCK - 1) // WBLOCK  # 7
SRCRANGE = 25000          # rows per src bucket (must be <= 32768 for int16)
NRANGE = (N + SRCRANGE - 1) // SRCRANGE  # 4
INDW = 96                 # indicator width
NLAYERS = int(os.environ.get("GCN_NLAYERS", str(L)))

F32 = np.float32


def _winw(w):
    return min(WIN, NP - w * WIN)


def preprocess(edge_index):
    """Host-side graph preprocessing. Returns (templates, per-core host data)."""
    src = np.concatenate([edge_index[0], np.arange(N, dtype=np.int64)])
    dst = np.concatenate([edge_index[1], np.arange(N, dtype=np.int64)])
    src = src.astype(np.int64)
    dst = dst.astype(np.int64)
    deg = np.bincount(dst, minlength=N).astype(F32)
    dinv = (1.0 / np.sqrt(deg)).astype(F32)  # deg >= 1 (self loops)

    cores = []
    counts = np.zeros((NCORES, NWIN, NRANGE), np.int64)
    for i in range(NCORES):
        m = (dst // NP) == i
        s = src[m]
        d = dst[m] - i * NP
        w = d // WIN
        r = s // SRCRANGE
        order = np.lexsort((d, r, w))
        s, d, w, r = s[order], d[order], w[order], r[order]
        key = w * NRANGE + r
        cnt = np.bincount(key, minlength=NWIN * NRANGE).reshape(NWIN, NRANGE)
        counts[i] = cnt
        cores.append((s, d, cnt))

    T = np.maximum((counts + 127) // 128, 0).max(axis=0)  # [NWIN, NRANGE] chunks
    nch = int(T.sum())
    ntot = nch * 128

    # shared program metadata: stream order = block -> range -> window
    # chunk global index base per (w, r); edge offset base per (b, r)
    chunk_base = np.zeros((NWIN, NRANGE), np.int64)
    call_edges = np.zeros((NBLK, NRANGE), np.int64)   # edges per gather call
    call_e0 = np.zeros((NBLK, NRANGE), np.int64)      # edge offset of call
    cb = 0
    for b in range(NBLK):
        for r in range(NRANGE):
            call_e0[b][r] = cb * 128
            for w in range(b * WBLOCK, min((b + 1) * WBLOCK, NWIN)):
                chunk_base[w][r] = cb
                cb += T[w][r]
            call_edges[b][r] = cb * 128 - call_e0[b][r]
    assert cb == nch

    per_core = []
    for i in range(NCORES):
        s, d, cnt = cores[i]
        idxs = np.zeros(ntot, np.int16)
        segs = np.full(ntot, -1.0, F32)
        offs = np.zeros(nch, np.int32)
        # group start offsets in the sorted per-core edge array
        gstart = np.zeros(NWIN * NRANGE + 1, np.int64)
        np.cumsum(cnt.reshape(-1), out=gstart[1:])
        for w in range(NWIN):
            winw = _winw(w)
            for r in range(NRANGE):
                n_real = cnt[w][r]
                t = T[w][r]
                if t == 0:
                    continue
                g0 = gstart[w * NRANGE + r]
                cbase = chunk_base[w][r]
                e0 = cbase * 128
                sg = s[g0:g0 + n_real]
                dg = d[g0:g0 + n_real]
                idxs[e0:e0 + n_real] = (sg - r * SRCRANGE).astype(np.int16)
                # per chunk: off and seg
                for j in range(t):
                    lo = j * 128
                    hi = min(lo + 128, n_real)
                    if hi <= lo:
                        offs[cbase + j] = 0
                        continue
                    dloc = dg[lo:hi] - w * WIN
                    off = int(dloc.min())
                    off = min(off, max(0, winw - INDW))
                    seg = dloc - off
                    assert seg.min() >= 0 and seg.max() < INDW, (
                        f"core {i} w {w} r {r} chunk {j}: seg range "
                        f"[{seg.min()},{seg.max()}]"
                    )
                    offs[cbase + j] = off
                    segs[e0 + lo:e0 + hi] = seg.astype(F32)
        eidx = np.tile(idxs.reshape(-1, 16).T, (8, 1))          # [128, ntot/16]
        eseg = np.ascontiguousarray(segs.reshape(nch, 128).T)   # [128, nch]
        eoff = offs.reshape(1, nch)
        per_core.append(dict(eidx=eidx, eseg=eseg, eoff=eoff,
                             dinv=dinv[i * NP:(i + 1) * NP]))
    meta = dict(T=T, nch=nch, ntot=ntot, chunk_base=chunk_base,
                call_edges=call_edges, call_e0=call_e0)
    return meta, per_core


def fold_weights(conv_w):
    betas = np.log(THETA / np.arange(1, L + 1, dtype=F32) + 1.0)
    wp = np.zeros((128, L * H), F32)
    eye = np.eye(H, dtype=F32)
    for l in range(L):
        wl = (1.0 - betas[l]) * eye + betas[l] * conv_w[l]
        wp[:H, l * H:(l + 1) * H] = wl
        wp[H:, l * H:(l + 1) * H] = wl
    return wp


def build_inputs(meta, per_core, inputs):
    """Per-core in_maps for the device program."""
    x_param = np.asarray(inputs["x_param"], F32)
    lin0_w = np.asarray(inputs["lin0_w"], F32)
    lin0_b = np.asarray(inputs["lin0_b"], F32)
    conv_w = np.asarray(inputs["conv_w"], F32)
    wp = fold_weights(conv_w)
    mlp_w0 = np.asarray(inputs["mlp_w0"], F32)
    mlp_w1 = np.asarray(inputs["mlp_w1"], F32)
    mlp_w2 = np.asarray(inputs["mlp_w2"], F32)
    out_w = np.asarray(inputs["out_w"], F32)

    iota96 = np.tile(np.arange(INDW, dtype=F32), (128, 1))
    iden2 = np.tile(np.eye(H, dtype=F32), (2, 1))               # [128, 64]
    lwt = np.zeros((125, 4, H), F32)
    for k in range(4):
        lwt[:, k, :] = lin0_w[k * 125:(k + 1) * 125, :]
    lb = np.zeros((H, 2), F32)
    lb[:, 0] = lin0_b
    lb[:, 1] = 0.1 * lin0_b
    w0d = np.tile(mlp_w0, (2, 1))                               # [128, 213]
    mw1a = mlp_w1[:128, :]
    mw1b = np.zeros((128, M2), F32)
    mw1b[:M1 - 128, :] = mlp_w1[128:, :]
    mw2 = np.zeros((3, 128, R), F32)
    mw2[0] = mlp_w2[:128]
    mw2[1] = mlp_w2[128:256]
    mw2[2, :M2 - 256] = mlp_w2[256:]
    owt = np.zeros((128, 4, C), F32)
    for mchunk in range(4):
        owt[:, mchunk, :] = out_w[mchunk * 128:(mchunk + 1) * 128, :]
    mb0 = np.zeros((128, 2), F32)
    mb0[:, 0] = np.asarray(inputs["mlp_b0"], F32)[:128]
    mb0[:M1 - 128, 1] = np.asarray(inputs["mlp_b0"], F32)[128:]
    mb1 = np.zeros((128, 3), F32)
    mb1[:, 0] = np.asarray(inputs["mlp_b1"], F32)[:128]
    mb1[:, 1] = np.asarray(inputs["mlp_b1"], F32)[128:256]
    mb1[:M2 - 256, 2] = np.asarray(inputs["mlp_b1"], F32)[256:]
    mb2 = np.zeros((128, 4), F32)
    for mchunk in range(4):
        mb2[:, mchunk] = np.asarray(inputs["mlp_b2"], F32)[mchunk * 128:(mchunk + 1) * 128]
    ob = np.tile(np.asarray(inputs["out_b"], F32), (128, 1))    # [128, 40]

    in_maps = []
    for i in range(NCORES):
        pc = per_core[i]
        dn = np.ones((128, 98), F32)
        dv = pc["dinv"]
        full = (NP // 128) * 128
        dn[:, :NP // 128] = dv[:full].reshape(-1, 128).T
        dn[:NP - full, NP // 128] = dv[full:]
        in_maps.append({
            "eidx": pc["eidx"], "eseg": pc["eseg"], "eoff": pc["eoff"],
            "xp": x_param[i * NP:(i + 1) * NP],
            "dinv_nm": dn, "iota96": iota96, "iden2": iden2,
            "wp": wp, "lwt": lwt, "lb": lb,
            "w0d": w0d, "mw1a": mw1a, "mw1b": mw1b,
            "mw2a": mw2[0], "mw2b": mw2[1], "mw2c": mw2[2],
            "owt": owt, "mb0": mb0, "mb1": mb1, "mb2": mb2, "ob": ob,
        })
    return in_maps


def build_program(meta, nlayers=NLAYERS):
    import concourse.bass as bass
    import concourse.bacc as bacc
    import concourse.mybir as mybir
    import concourse.tile as tile
    from concourse.bass import ds
    from concourse.masks import make_identity

    T = meta["T"]
    nch = meta["nch"]
    ntot = meta["ntot"]
    chunk_base = meta["chunk_base"]
    call_edges = meta["call_edges"]
    call_e0 = meta["call_e0"]
    PE = mybir.EngineType.PE
    f32 = mybir.dt.float32
    AF = mybir.ActivationFunctionType
    ALU = mybir.AluOpType

    nc = bacc.Bacc("TRN2", target_bir_lowering=False, debug=False,
                   num_devices=NCORES)
    # ---- I/O ----
    eidx_in = nc.declare_dram_parameter("eidx", [128, ntot // 16], mybir.dt.int16, isOutput=False)
    eseg_in = nc.declare_dram_parameter("eseg", [128, nch], f32, isOutput=False)
    eoff_in = nc.declare_dram_parameter("eoff", [1, nch], mybir.dt.int32, isOutput=False)
    xp_in = nc.declare_dram_parameter("xp", [NP, F], f32, isOutput=False)
    dinv_in = nc.declare_dram_parameter("dinv_nm", [128, 98], f32, isOutput=False)
    iota_in = nc.declare_dram_parameter("iota96", [128, INDW], f32, isOutput=False)
    iden2_in = nc.declare_dram_parameter("iden2", [128, H], f32, isOutput=False)
    wp_in = nc.declare_dram_parameter("wp", [128, L * H], f32, isOutput=False)
    lwt_in = nc.declare_dram_parameter("lwt", [125, 4 * H], f32, isOutput=False)
    lb_in = nc.declare_dram_parameter("lb", [H, 2], f32, isOutput=False)
    w0d_in = nc.declare_dram_parameter("w0d", [128, M1], f32, isOutput=False)
    mw1a_in = nc.declare_dram_parameter("mw1a", [128, M2], f32, isOutput=False)
    mw1b_in = nc.declare_dram_parameter("mw1b", [128, M2], f32, isOutput=False)
    mw2a_in = nc.declare_dram_parameter("mw2a", [128, R], f32, isOutput=False)
    mw2b_in = nc.declare_dram_parameter("mw2b", [128, R], f32, isOutput=False)
    mw2c_in = nc.declare_dram_parameter("mw2c", [128, R], f32, isOutput=False)
    owt_in = nc.declare_dram_parameter("owt", [128, 4 * C], f32, isOutput=False)
    mb0_in = nc.declare_dram_parameter("mb0", [128, 2], f32, isOutput=False)
    mb1_in = nc.declare_dram_parameter("mb1", [128, 3], f32, isOutput=False)
    mb2_in = nc.declare_dram_parameter("mb2", [128, 4], f32, isOutput=False)
    ob_in = nc.declare_dram_parameter("ob", [128, C], f32, isOutput=False)
    out_d = nc.declare_dram_parameter("out", [NP, C], f32, isOutput=True)

    xfull = [nc.dram_tensor(f"xfull{p}", [N, H], f32, addr_space="Shared")
             for p in range(2)]
    slab = [nc.dram_tensor(f"slab{p}", [NP, H], f32) for p in range(2)]

    NJ = 98  # node-major 128-subchunks (last = 84 rows)

    def xfin_loc(w):
        return 64 * (w % 2), (w // 2) * WIN

    with tile.TileContext(nc) as tc:
        with (
            tc.tile_pool(name="cst", bufs=1) as cst,
        ):
            # ---- resident tiles ----
            eidx = cst.tile([128, ntot // 16], mybir.dt.int16)
            eseg = cst.tile([128, nch], f32)
            eoff = cst.tile([1, nch], mybir.dt.int32)
            dinv_nm = cst.tile([128, 98], f32)
            iota96 = cst.tile([128, INDW], f32)
            iden2 = cst.tile([128, H], f32)
            iden = cst.tile([128, 128], f32)
            wp = cst.tile([128, L * H], f32)
            wdx = cst.tile([128, NP], f32)     # rows 0:64 = 0.1*x0, 64:128 = 0.9*dinv
            x_fin = cst.tile([128, ((NWIN + 1) // 2) * WIN], f32)
            lwt = cst.tile([125, 4 * H], f32)
            lb = cst.tile([H, 2], f32)
            ones09 = cst.tile([128, H], f32)

            nc.sync.dma_start(eidx[:], eidx_in[:])
            nc.sync.dma_start(eseg[:], eseg_in[:])
            nc.sync.dma_start(eoff[:], eoff_in[:])
            nc.sync.dma_start(dinv_nm[:], dinv_in[:])
            nc.sync.dma_start(iota96[:], iota_in[:])
            nc.sync.dma_start(iden2[:], iden2_in[:])
            nc.sync.dma_start(wp[:], wp_in[:])
            nc.sync.dma_start(lwt[:], lwt_in[:])
            nc.sync.dma_start(lb[:], lb_in[:])
            make_identity(nc, iden[:])
            nc.gpsimd.memset(ones09[:], 0.9)

            def tail_window(w, wpool, stpool, out_par, psC):
                """x_fin window -> transpose -> dinv scale -> slab[out_par]."""
                if os.environ.get("GCN_NO_TAIL"):
                    return
                xb, xc = xfin_loc(w)
                winw = _winw(w)
                stage = stpool.tile([128, 4 * H], f32, tag="stage")
                ng = (winw + 127) // 128
                for g in range(ng):
                    gn = min(128, winw - g * 128)
                    pst = psC.tile([128, H], f32, space="PSUM", tag="psC")
                    nc.tensor.transpose(
                        pst[:gn, :],
                        x_fin[xb:xb + H, xc + g * 128: xc + g * 128 + gn],
                        iden2[xb:xb + H, :],
                    )
                    nc.scalar.activation(
                        stage[:gn, g * H:(g + 1) * H], pst[:gn, :], AF.Copy,
                        scale=dinv_nm[:gn, (4 * w + g):(4 * w + g) + 1],
                    )
                base = w * WIN
                if winw == WIN:
                    nc.sync.dma_start(
                        slab[out_par][base:base + WIN, :].rearrange(
                            "(g p) f -> p g f", p=128),
                        stage[:].rearrange("p (g f) -> p g f", f=H),
                    )
                else:
                    for g in range(ng):
                        gn = min(128, winw - g * 128)
                        nc.sync.dma_start(
                            slab[out_par][base + g * 128:base + g * 128 + gn, :],
                            stage[:gn, g * H:(g + 1) * H],
                        )

            def allgather(out_par):
                if os.environ.get("GCN_NO_CC"):
                    nc.sync.dma_start(xfull[out_par][:NP, :], slab[out_par][:])
                    return
                nc.gpsimd.collective_compute(
                    "AllGather", ALU.bypass,
                    replica_groups=[list(range(NCORES))],
                    ins=[slab[out_par][:]],
                    outs=[xfull[out_par][:]],
                )

            # ================= init: wd, x0, first slab =================
            if os.environ.get("GCN_NO_INIT"):
                nc.gpsimd.memset(wdx[:], 0.1)
                nc.gpsimd.memset(x_fin[:], 0.1)
            else:
              with (
                tc.tile_pool(name="initp", bufs=2) as initp,
                  tc.tile_pool(name="initw", bufs=3) as initw,
                  tc.tile_pool(name="ipsB", bufs=2, space="PSUM") as psB,
                  tc.tile_pool(name="ipsC", bufs=2, space="PSUM") as psC,
              ):
                  # wd = 0.9 * dinv broadcast over 64 partitions -> wdx[64:]
                  for j in range(NJ):
                      gn = min(128, NP - j * 128)
                      diag = initw.tile([128, 128], f32, tag="diag")
                      nc.vector.tensor_scalar_mul(
                          diag[:], iden[:], dinv_nm[:, j:j + 1])
                      pw = psB.tile([H, 128], f32, space="PSUM", tag="psB")
                      nc.tensor.matmul(pw[:, :gn], lhsT=ones09[:],
                                       rhs=diag[:, :gn], start=True, stop=True)
                      nc.scalar.activation(
                          wdx[H:, j * 128:j * 128 + gn], pw[:, :gn], AF.Copy)
                  # x0 = relu(xp @ lin0_w + b)
                  for c in range(NWIN):
                      winw = _winw(c)
                      ng = (winw + 127) // 128
                      xpt = initp.tile([128, 4 * F], f32, tag="xpt")
                      base = c * WIN
                      if winw == WIN:
                          nc.sync.dma_start(
                              xpt[:].rearrange("p (g f) -> p g f", f=F),
                              xp_in[base:base + WIN, :].rearrange(
                                  "(g p) f -> p g f", p=128),
                          )
                      else:
                          for g in range(ng):
                              gn = min(128, winw - g * 128)
                              nc.sync.dma_start(
                                  xpt[:gn, g * F:(g + 1) * F],
                                  xp_in[base + g * 128:base + g * 128 + gn, :],
                              )
                      px0 = psB.tile([H, WIN], f32, space="PSUM", tag="psB")
                      for k in range(4):
                          xt = initw.tile([125, WIN], f32, tag=f"xt{k}")
                          for g in range(ng):
                              gn = min(128, winw - g * 128)
                              ptr = psC.tile([128, 128], f32, space="PSUM",
                                             tag="psC")
                              nc.tensor.transpose(
                                  ptr[:125, :gn],
                                  xpt[:gn, g * F + 125 * k: g * F + 125 * (k + 1)],
                                  iden[:gn, :gn],
                              )
                              nc.scalar.activation(
                                  xt[:, g * 128:g * 128 + gn], ptr[:125, :gn],
                                  AF.Copy)
                          nc.tensor.matmul(
                              px0[:, :winw], lhsT=lwt[:, k * H:(k + 1) * H],
                              rhs=xt[:, :winw], start=(k == 0), stop=(k == 3))
                      xb, xc = xfin_loc(c)
                      nc.scalar.activation(
                          x_fin[xb:xb + H, xc:xc + winw], px0[:, :winw],
                          AF.Relu, bias=lb[:, 0:1])
                      nc.scalar.activation(
                          wdx[:H, base:base + winw], px0[:, :winw],
                          AF.Relu, bias=lb[:, 1:2], scale=0.1)
                  for w in range(NWIN):
                      tail_window(w, initw, initw, 0, psC)
                  allgather(0)

            # ================= layers =================
            with (
                tc.tile_pool(name="gp", bufs=2) as gpool,
                tc.tile_pool(name="ip", bufs=2) as ipool,
                tc.tile_pool(name="hp", bufs=2) as hpool,
                tc.tile_pool(name="stp", bufs=2) as stpool,
                tc.tile_pool(name="psA", bufs=4, space="PSUM") as psA,
                tc.tile_pool(name="psB", bufs=2, space="PSUM") as psB,
                tc.tile_pool(name="psC", bufs=2, space="PSUM") as psC,
            ):
                cbmax = int(call_edges.max()) // 128
                tmax = int(T.max())
                GPBUFS, IPBUFS, PSABUFS = 2, 2, 4
                g_hist = []
                g_hist_idx = {}
                ind_hist = []
                psw_hist = []
                maxblk = int(os.environ.get("GCN_MAXBLK", str(NBLK)))
                for lay in range(nlayers):
                    par = lay % 2
                    for b in range(min(NBLK, maxblk)):
                        gt = {}
                        gt_last = {}
                        for r in range(NRANGE):
                            ce = int(call_edges[b][r])
                            if ce == 0:
                                continue
                            e0 = int(call_e0[b][r])
                            g = gpool.tile([128, cbmax * H], f32, tag="gath")
                            rows = min(SRCRANGE, N - r * SRCRANGE)
                            gref = nc.gpsimd.dma_gather(
                                out_ap=g[:, :(ce // 128) * H].rearrange(
                                    "p (c d) -> p c d", d=H),
                                in_ap=xfull[par][r * SRCRANGE:r * SRCRANGE + rows, :],
                                idxs_ap=eidx[:, e0 // 16:(e0 + ce) // 16],
                                num_idxs=ce,
                                num_idxs_reg=ce,
                                elem_size=H,
                                single_packet=False,
                            )
                            if len(g_hist) >= GPBUFS and g_hist[-GPBUFS] is not None:
                                tile.add_dep_helper(gref.ins, g_hist[-GPBUFS].ins,
                                                    sync=False)
                            g_hist.append(None)  # placeholder, set after matmuls
                            g_hist_idx[id(g)] = len(g_hist) - 1
                            gt[r] = (g, e0 // 128)
                        wlist = list(range(b * WBLOCK,
                                           min((b + 1) * WBLOCK, NWIN)))
                        pswt = {}
                        wfirst = {w: True for w in wlist}
                        wleft = {w: sum(int(T[w][r]) for r in range(NRANGE))
                                 for w in wlist}
                        for r in range(NRANGE):
                            for w in wlist:
                                t = int(T[w][r])
                                if t == 0:
                                    continue
                                if w not in pswt:
                                    pswt[w] = psA.tile([H, WIN], f32,
                                                       space="PSUM", tag="psw",
                                                       name=f"psw{w}")
                                psw = pswt[w]
                                c0 = int(chunk_base[w][r])
                                g, gchunk0 = gt[r]
                                jj0 = c0 - gchunk0
                                ind = ipool.tile([128, tmax * INDW], f32,
                                                 tag="ind")
                                ie = None if os.environ.get("GCN_NO_IE") else nc.vector.tensor_tensor(
                                    out=ind[:, :t * INDW].rearrange(
                                        "p (c s) -> p c s", s=INDW),
                                    in0=iota96[:].rearrange(
                                        "p (u s) -> p u s", u=1
                                    ).to_broadcast([128, t, INDW]),
                                    in1=eseg[:, c0:c0 + t].rearrange(
                                        "p (c u) -> p c u", u=1
                                    ).to_broadcast([128, t, INDW]),
                                    op=ALU.is_equal,
                                )
                                if ie is not None and len(ind_hist) >= IPBUFS and ind_hist[-IPBUFS] is not None:
                                    tile.add_dep_helper(
                                        ie.ins, ind_hist[-IPBUFS].ins, sync=False)
                                ie_mm = None
                                if os.environ.get("GCN_STATIC_OFF"):
                                    offv = [0] * t
                                else:
                                    _, offv = nc.values_load_multi_w_load_instructions(
                                        eoff[0:1, c0:c0 + t], engines=[PE],
                                        min_val=0, max_val=WIN - INDW,
                                        skip_runtime_bounds_check=True,
                                    )
                                for j in range(t):
                                    wleft[w] -= 1
                                    if os.environ.get("GCN_NO_MM"):
                                        continue
                                    mm = nc.tensor.matmul(
                                        psw[:, ds(offv[j], INDW)],
                                        lhsT=g[:, (jj0 + j) * H:(jj0 + j + 1) * H],
                                        rhs=ind[:, j * INDW:(j + 1) * INDW],
                                        start=wfirst[w],
                                        stop=(wleft[w] == 0),
                                    )
                                    if wfirst[w] and len(psw_hist) >= PSABUFS:
                                        tile.add_dep_helper(
                                            mm.ins, psw_hist[-PSABUFS].ins,
                                            sync=False)
                                    wfirst[w] = False
                                    ie_mm = mm
                                ind_hist.append(ie_mm)
                                gt_last[r] = ie_mm
                        for w in wlist:
                            if os.environ.get("GCN_NO_HW2"):
                                break
                            winw = _winw(w)
                            psw = pswt[w]
                            # h = psw * wd + x0s ; x = relu(h @ W'l)
                            h = hpool.tile([H, WIN], f32, tag="h")
                            hh = hpool.tile([H, WIN], f32, tag="hh")
                            hm = nc.vector.tensor_tensor(
                                h[:, :winw], psw[:, :winw],
                                wdx[H:, w * WIN:w * WIN + winw], op=ALU.mult)
                            psw_hist.append(hm)
                            nc.vector.tensor_tensor(
                                hh[:, :winw], h[:, :winw],
                                wdx[:H, w * WIN:w * WIN + winw], op=ALU.add)
                            ps2 = psB.tile([H, WIN], f32, space="PSUM",
                                           tag="psB")
                            nc.tensor.matmul(
                                ps2[:, :winw],
                                lhsT=wp[:H, lay * H:(lay + 1) * H],
                                rhs=hh[:, :winw], start=True, stop=True)
                            xb, xc = xfin_loc(w)
                            nc.scalar.activation(
                                x_fin[xb:xb + H, xc:xc + winw],
                                ps2[:, :winw], AF.Relu)
                        for r in range(NRANGE):
                            if r in gt:
                                g, _ = gt[r]
                                g_hist[g_hist_idx[id(g)]] = gt_last.get(r)
                        if lay < nlayers - 1:
                            for w in range(b * WBLOCK,
                                           min((b + 1) * WBLOCK, NWIN)):
                                tail_window(w, hpool, stpool, 1 - par, psC)
                    if lay < nlayers - 1:
                        allgather(1 - par)

            # ================= MLP head =================
            if os.environ.get("GCN_NO_MLP"):
                nc.sync.dma_start(out_d[:128, :40], x_fin[:128, :40])
            elif True:
              with (
                tc.tile_pool(name="mh", bufs=2) as mh,
                  tc.tile_pool(name="mw", bufs=1) as mw,
                  tc.tile_pool(name="mpsA", bufs=4, space="PSUM") as psA,
                  tc.tile_pool(name="mpsB", bufs=2, space="PSUM") as psB,
                  tc.tile_pool(name="mpsC", bufs=2, space="PSUM") as psC,
              ):
                  w0d = mw.tile([128, M1], f32)
                  mw1a = mw.tile([128, M2], f32)
                  mw1b = mw.tile([128, M2], f32)
                  mw2a = mw.tile([128, R], f32)
                  mw2b = mw.tile([128, R], f32)
                  mw2c = mw.tile([128, R], f32)
                  owt = mw.tile([128, 4 * C], f32)
                  mb0 = mw.tile([128, 2], f32)
                  mb1 = mw.tile([128, 3], f32)
                  mb2 = mw.tile([128, 4], f32)
                  ob = mw.tile([128, C], f32)
                  nc.sync.dma_start(w0d[:], w0d_in[:])
                  nc.sync.dma_start(mw1a[:], mw1a_in[:])
                  nc.sync.dma_start(mw1b[:], mw1b_in[:])
                  nc.sync.dma_start(mw2a[:], mw2a_in[:])
                  nc.sync.dma_start(mw2b[:], mw2b_in[:])
                  nc.sync.dma_start(mw2c[:], mw2c_in[:])
                  nc.sync.dma_start(owt[:], owt_in[:])
                  nc.sync.dma_start(mb0[:], mb0_in[:])
                  nc.sync.dma_start(mb1[:], mb1_in[:])
                  nc.sync.dma_start(mb2[:], mb2_in[:])
                  nc.sync.dma_start(ob[:], ob_in[:])
                  for c in range(NWIN):
                      winw = _winw(c)
                      xb, xc = xfin_loc(c)
                      xin = x_fin[xb:xb + H, xc:xc + winw]
                      # h1 = relu(x @ w0 + b0): [213, winw]
                      h1a = mh.tile([128, WIN], f32, tag="h1a")
                      h1b = mh.tile([M1 - 128, WIN], f32, tag="h1b")
                      p1 = psB.tile([128, WIN], f32, space="PSUM", tag="psB")
                      nc.tensor.matmul(p1[:, :winw], lhsT=w0d[xb:xb + H, :128],
                                       rhs=xin, start=True, stop=True)
                      nc.scalar.activation(h1a[:, :winw], p1[:, :winw], AF.Relu,
                                           bias=mb0[:, 0:1])
                      p1b = psB.tile([M1 - 128, WIN], f32, space="PSUM",
                                     tag="psB")
                      nc.tensor.matmul(p1b[:, :winw],
                                       lhsT=w0d[xb:xb + H, 128:M1],
                                       rhs=xin, start=True, stop=True)
                      nc.scalar.activation(h1b[:, :winw], p1b[:, :winw], AF.Relu,
                                           bias=mb0[:M1 - 128, 1:2])
                      # h2 = relu(h1 @ w1 + b1): [362, winw]
                      h2 = []
                      for mchunk in range(3):
                          mn = min(128, M2 - mchunk * 128)
                          p2 = psA.tile([128, WIN], f32, space="PSUM", tag="psw")
                          nc.tensor.matmul(
                              p2[:mn, :winw],
                              lhsT=mw1a[:, mchunk * 128:mchunk * 128 + mn],
                              rhs=h1a[:, :winw], start=True, stop=False)
                          nc.tensor.matmul(
                              p2[:mn, :winw],
                              lhsT=mw1b[:M1 - 128, mchunk * 128:mchunk * 128 + mn],
                              rhs=h1b[:M1 - 128, :winw], start=False, stop=True)
                          h2t = mh.tile([128, WIN], f32, tag=f"h2_{mchunk}")
                          nc.scalar.activation(h2t[:mn, :winw], p2[:mn, :winw],
                                               AF.Relu,
                                               bias=mb1[:mn, mchunk:mchunk + 1])
                          h2.append(h2t)
                      # h3 = h2 @ w2 + b2 (no relu): [512, winw]
                      h3 = []
                      w2t = [mw2a, mw2b, mw2c]
                      kn = [128, 128, M2 - 256]
                      for mchunk in range(4):
                          p3 = psA.tile([128, WIN], f32, space="PSUM", tag="psw")
                          for k in range(3):
                              nc.tensor.matmul(
                                  p3[:, :winw],
                                  lhsT=w2t[k][:kn[k],
                                              mchunk * 128:(mchunk + 1) * 128],
                                  rhs=h2[k][:kn[k], :winw],
                                  start=(k == 0), stop=(k == 2))
                          h3t = mh.tile([128, WIN], f32, tag=f"h3_{mchunk}")
                          nc.scalar.activation(h3t[:, :winw], p3[:, :winw],
                                               AF.Identity,
                                               bias=mb2[:, mchunk:mchunk + 1])
                          h3.append(h3t)
                      # out = h3 @ out_w + out_b: node-major [winw, 40]
                      ost = mh.tile([128, 4 * C], f32, tag="ost")
                      ng = (winw + 127) // 128
                      for g in range(ng):
                          gn = min(128, winw - g * 128)
                          po = psC.tile([128, C], f32, space="PSUM", tag="psC")
                          for k in range(4):
                              nc.tensor.matmul(
                                  po[:gn, :],
                                  lhsT=h3[k][:, g * 128:g * 128 + gn],
                                  rhs=owt[:, k * C:(k + 1) * C],
                                  start=(k == 0), stop=(k == 3))
                          nc.vector.tensor_tensor(
                              ost[:gn, g * C:(g + 1) * C], po[:gn, :],
                              ob[:gn, :], op=ALU.add)
                      base = c * WIN
                      if winw == WIN:
                          nc.sync.dma_start(
                              out_d[base:base + WIN, :].rearrange(
                                  "(g p) f -> p g f", p=128),
                              ost[:].rearrange("p (g f) -> p g f", f=C),
                          )
                      else:
                          for g in range(ng):
                              gn = min(128, winw - g * 128)
                              nc.sync.dma_start(
                                  out_d[base + g * 128:base + g * 128 + gn, :],
                                  ost[:gn, g * C:(g + 1) * C],
                              )
    nc.finalize()
    return nc


_CACHE = {}


def _get_program(meta, nlayers):
    key = (meta["nch"], nlayers, meta["T"].tobytes())
    if key not in _CACHE:
        _CACHE[key] = build_program(meta, nlayers)
    return _CACHE[key]


_LAST_EXEC_NS = None


def kernel(**inputs):
    global _LAST_EXEC_NS
    from concourse.bass_utils import run_bass_kernel_spmd
    edge_index = np.asarray(inputs["edge_index"])
    meta, per_core = preprocess(edge_index)
    in_maps = build_inputs(meta, per_core, inputs)
    nc = _get_program(meta, NLAYERS)
    kw = {}
    if os.environ.get("GCN_TRACE"):
        kw = dict(trace=True, tmpdir=os.environ.get("GCN_TRACE_DIR") or None)
    res = run_bass_kernel_spmd(nc, in_maps, list(range(NCORES)), **kw)
    if res.exec_time_ns is not None:
        _LAST_EXEC_NS = res.exec_time_ns
    out = np.concatenate([res.results[i]["out"] for i in range(NCORES)], axis=0)
    return out

